# revision 2
# baseline (speedup 1.0000x reference)
"""Trainium2 Bass kernel for DirectInterpGNN message passing.

Math (per reference):
    num_v  = sum_{e: src_e=v} A_e
    den_v  = sum_{e: src_e=v} A_e*S_e*v_e
    f_v    = (C_v - 1) * (num_v/den_v) / A_ii_v
    w_e    = A_e * f_{src_e}

Distribution: edges split contiguously across 8 NeuronCores (2M edges each),
vertex table replicated. Each core computes partial per-vertex sums via
PE-deduplicated indirect scatter-add into K replicated DRAM tables, the
partials are AllReduced across the 8 cores, each core computes the per-vertex
factor f, then re-walks its edges gathering f[src] to produce w.

Per-128-edge-tile scatter correctness: indices within a tile are deduplicated
with a PE selection-matrix (duplicate edges' values are pre-summed by a
matmul and only the first occurrence row carries a real index; duplicates are
routed to a trash row). Tiles round-robin over K independent table replicas so
in-flight scatter-adds never touch the same replica concurrently (Tile
serializes same-replica writers); replicas are summed at the end.
"""
import sys
sys.path.insert(0, '/opt/trn_rl_repo')
sys.path.insert(0, '/root/.axon_site/_ro/trn_rl_repo')

import numpy as np

P = 128
N_CORES = 8

# full-size problem constants (hardcoded per task spec)
E_FULL = 16_000_000
N_VERT = 500_000


def _params(e_core, n_vert, u_tiles, n_chunks):
    cols = -(-n_vert // P // 2) * 2 + 2          # vertices per partition (even, +pad)
    while (cols * P) % (2 * n_chunks) or cols % n_chunks:
        cols += 2
    npad = cols * P
    assert e_core % (P * u_tiles) == 0
    return dict(
        E_CORE=e_core, NPAD=npad, COLS=cols, TRASH=npad - 1,
        U=u_tiles, OUTER=e_core // (P * u_tiles), NCHUNK=n_chunks,
        CHW=2 * cols // n_chunks,               # chunk width in the [P, 2*COLS] view
        FCW=cols // n_chunks,                   # chunk width in the [P, COLS] f view
    )


FULL = dict(e_core=E_FULL // N_CORES, n_vert=N_VERT, u_tiles=25, n_chunks=4)
K_REP = 16


def build_kernel(e_core, n_vert, u_tiles, n_chunks, n_cores=N_CORES,
                 use_collective=True):
    import concourse.bass as bass
    import concourse.bacc as bacc
    import concourse.mybir as mybir
    import concourse.tile as tile
    from concourse.masks import make_identity

    p = _params(e_core, n_vert, u_tiles, n_chunks)
    E_CORE, NPAD, COLS, TRASH = p["E_CORE"], p["NPAD"], p["COLS"], p["TRASH"]
    U, OUTER, NCHUNK, CHW, FCW = p["U"], p["OUTER"], p["NCHUNK"], p["CHW"], p["FCW"]
    TE = P * U                                   # edges per outer iteration
    f32 = mybir.dt.float32
    i32 = mybir.dt.int32

    nc = bacc.Bacc("TRN2", target_bir_lowering=False, debug=False,
                   num_devices=n_cores)
    src = nc.dram_tensor("src", [OUTER, P, U], i32, kind="ExternalInput")
    attr = nc.dram_tensor("attr", [OUTER, P, 3 * U], f32, kind="ExternalInput")
    vattr = nc.dram_tensor("vattr", [NPAD, 2], f32, kind="ExternalInput")
    w = nc.dram_tensor("w", [OUTER, P, U], f32, kind="ExternalOutput")

    with tile.TileContext(nc) as tc:
        with (tc.tile_pool(name="const", bufs=1) as cpool,
              tc.tile_pool(name="work", bufs=2) as wpool,
              tc.tile_pool(name="mwork", bufs=3) as mpool,
              tc.tile_pool(name="psT", bufs=3, space="PSUM") as psT_pool,
              tc.tile_pool(name="psS", bufs=2, space="PSUM") as psS_pool,
              tc.tile_pool(name="dram", bufs=1, space="DRAM") as dpool):

            ident = cpool.tile([P, P], f32)
            make_identity(nc, ident[:])
            # strict lower-triangular mask: LT[p, q] = 1 if q < p else 0
            lt = cpool.tile([P, P], f32)
            iot_q = cpool.tile([P, P], i32)
            nc.gpsimd.iota(iot_q[:], pattern=[[1, P]], base=0,
                           channel_multiplier=0)
            iot_p = cpool.tile([P, P], i32)
            nc.gpsimd.iota(iot_p[:], pattern=[[0, P]], base=0,
                           channel_multiplier=1)
            iot_qf = cpool.tile([P, P], f32)
            nc.vector.tensor_copy(iot_qf[:], iot_q[:])
            iot_pf = cpool.tile([P, P], f32)
            nc.vector.tensor_copy(iot_pf[:], iot_p[:])
            nc.vector.tensor_tensor(
                out=lt[:], in0=iot_qf[:], in1=iot_pf[:],
                op=mybir.AluOpType.is_lt)

            reps = []
            for k in range(K_REP):
                rk = dpool.tile([NPAD, 2], f32, name=f"rep{k}")
                reps.append(rk)
            zt = cpool.tile([P, CHW], f32)
            nc.vector.memset(zt[:], 0.0)
            for k in range(K_REP):
                rv = reps[k][:].rearrange("(p c) v -> p (c v)", p=P)
                for ch in range(NCHUNK):
                    nc.sync.dma_start(rv[:, ch * CHW:(ch + 1) * CHW], zt[:])

            # ---------------- phase A: dedup + scatter-add ----------------
            with tc.For_i(0, OUTER, 1) as i:
                src_t = wpool.tile([P, U], i32)
                nc.sync.dma_start(src_t[:], src[i, :, :])
                attr_t = wpool.tile([P, 3 * U], f32)
                nc.sync.dma_start(attr_t[:], attr[i, :, :])
                at3 = attr_t[:].rearrange("p (j v) -> p j v", v=3)
                a_v = at3[:, :, 0]
                s_v = at3[:, :, 1]
                v_v = at3[:, :, 2]

                idxf = wpool.tile([P, U], f32)
                nc.vector.tensor_copy(idxf[:], src_t[:])
                m_sl = wpool.tile([P, U], f32)
                nc.vector.tensor_tensor(
                    out=m_sl[:], in0=a_v, in1=s_v, op=mybir.AluOpType.mult)
                nc.vector.tensor_tensor(
                    out=m_sl[:], in0=m_sl[:], in1=v_v, op=mybir.AluOpType.mult)
                paired = wpool.tile([P, 2 * U], f32)
                pr3 = paired[:].rearrange("p (j v) -> p j v", v=2)
                nc.vector.tensor_copy(pr3[:, :, 0], a_v)
                nc.vector.tensor_copy(pr3[:, :, 1], m_sl[:])

                occ = wpool.tile([P, U], f32)
                psumS = psS_pool.tile([P, 2 * U], f32)
                for j in range(U):
                    col = idxf[:, j:j + 1]
                    psumT = psT_pool.tile([P, P], f32, tag="psT")
                    nc.tensor.transpose(
                        out=psumT[:], in_=col.to_broadcast([P, P]),
                        identity=ident[:])
                    idxT = mpool.tile([P, P], f32, tag="idxT")
                    nc.vector.tensor_copy(idxT[:], psumT[:])
                    msel = mpool.tile([P, P], f32, tag="msel")
                    nc.vector.tensor_tensor(
                        out=msel[:], in0=col.to_broadcast([P, P]), in1=idxT[:],
                        op=mybir.AluOpType.is_equal)
                    scrap = mpool.tile([P, P], f32, tag="scrap")
                    nc.vector.scalar_tensor_tensor(
                        out=scrap[:], in0=msel[:], scalar=1.0, in1=lt[:],
                        op0=mybir.AluOpType.mult, op1=mybir.AluOpType.mult,
                        accum_out=occ[:, j:j + 1])
                    nc.tensor.matmul(
                        out=psumS[:, 2 * j:2 * j + 2], lhsT=msel[:],
                        rhs=pr3[:, j, :], start=True, stop=True)

                svals = wpool.tile([P, 2 * U], f32)
                nc.vector.tensor_copy(svals[:], psumS[:])
                mask = wpool.tile([P, U], f32)
                nc.vector.tensor_scalar(
                    out=mask[:], in0=occ[:], scalar1=0.0, scalar2=None,
                    op0=mybir.AluOpType.is_equal)
                sidxf = wpool.tile([P, U], f32)
                nc.vector.scalar_tensor_tensor(
                    out=sidxf[:], in0=idxf[:], scalar=float(-TRASH), in1=mask[:],
                    op0=mybir.AluOpType.add, op1=mybir.AluOpType.mult)
                nc.vector.tensor_scalar(
                    out=sidxf[:], in0=sidxf[:], scalar1=float(TRASH), scalar2=None,
                    op0=mybir.AluOpType.add)
                sidx = wpool.tile([P, U], i32)
                nc.vector.tensor_copy(sidx[:], sidxf[:])
                sv3 = svals[:].rearrange("p (j v) -> p j v", v=2)
                for j in range(U):
                    nc.gpsimd.indirect_dma_start(
                        out=reps[j % K_REP][:],
                        out_offset=bass.IndirectOffsetOnAxis(
                            ap=sidx[:, j:j + 1], axis=0),
                        in_=sv3[:, j, :],
                        in_offset=None,
                        compute_op=mybir.AluOpType.add)

            # ---------------- merge replicas ----------------
            partial = dpool.tile([P, 2 * COLS], f32)
            for ch in range(NCHUNK):
                sl = slice(ch * CHW, (ch + 1) * CHW)
                acc = mpool.tile([P, CHW], f32, tag="acc")
                nc.sync.dma_start(
                    acc[:], reps[0][:].rearrange("(p c) v -> p (c v)", p=P)[:, sl])
                for k in range(1, K_REP):
                    tk = mpool.tile([P, CHW], f32, tag="tk")
                    nc.sync.dma_start(
                        tk[:],
                        reps[k][:].rearrange("(p c) v -> p (c v)", p=P)[:, sl])
                    nc.vector.tensor_tensor(
                        out=acc[:], in0=acc[:], in1=tk[:],
                        op=mybir.AluOpType.add)
                nc.sync.dma_start(partial[:, sl], acc[:])

            # ---------------- all-reduce ----------------
            if use_collective:
                ar_out = dpool.tile([P, 2 * COLS], f32, name="ar_out")
                nc.gpsimd.collective_compute(
                    "AllReduce", mybir.AluOpType.add,
                    replica_groups=[list(range(n_cores))],
                    ins=[partial.opt()],
                    outs=[ar_out.opt()])
                table = ar_out
            else:
                table = partial

            # ---------------- vertex math: f = (C-1)*num/den/A_ii ----------
            f_tab = dpool.tile([NPAD, 1], f32)
            fv = f_tab[:].rearrange("(p c) v -> p (c v)", p=P)
            for ch in range(NCHUNK):
                sl = slice(ch * CHW, (ch + 1) * CHW)
                tt = mpool.tile([P, CHW], f32, tag="tt")
                nc.sync.dma_start(tt[:], table[:, sl])
                va = mpool.tile([P, CHW], f32, tag="va")
                nc.sync.dma_start(
                    va[:], vattr[:].rearrange("(p c) v -> p (c v)", p=P)[:, sl])
                tt3 = tt[:].rearrange("p (c v) -> p c v", v=2)
                va3 = va[:].rearrange("p (c v) -> p c v", v=2)
                fch = mpool.tile([P, FCW], f32, tag="fch")
                dsafe = mpool.tile([P, FCW], f32, tag="dsafe")
                # den==0 only for vertices with no incident edges (num==0 too,
                # so f becomes 0 instead of NaN)
                nc.vector.tensor_scalar(
                    out=dsafe[:], in0=tt3[:, :, 1], scalar1=0.0, scalar2=None,
                    op0=mybir.AluOpType.is_equal)
                nc.vector.tensor_tensor(
                    out=dsafe[:], in0=dsafe[:], in1=tt3[:, :, 1],
                    op=mybir.AluOpType.add)
                # fold A_ii into the denominator, then one reciprocal
                nc.vector.tensor_tensor(
                    out=dsafe[:], in0=dsafe[:], in1=va3[:, :, 0],
                    op=mybir.AluOpType.mult)
                nc.vector.reciprocal(out=dsafe[:], in_=dsafe[:])
                nc.vector.tensor_tensor(
                    out=fch[:], in0=tt3[:, :, 0], in1=dsafe[:],
                    op=mybir.AluOpType.mult)
                cm1 = mpool.tile([P, FCW], f32, tag="cm1")
                nc.vector.tensor_scalar(
                    out=cm1[:], in0=va3[:, :, 1], scalar1=-1.0, scalar2=None,
                    op0=mybir.AluOpType.add)
                nc.vector.tensor_tensor(
                    out=fch[:], in0=fch[:], in1=cm1[:],
                    op=mybir.AluOpType.mult)
                nc.sync.dma_start(fv[:, ch * FCW:(ch + 1) * FCW], fch[:])

            # ---------------- phase C: w = A * f[src] ----------------
            with tc.For_i(0, OUTER, 1) as i:
                src_t2 = wpool.tile([P, U], i32)
                nc.sync.dma_start(src_t2[:], src[i, :, :])
                attr_t2 = wpool.tile([P, 3 * U], f32)
                nc.sync.dma_start(attr_t2[:], attr[i, :, :])
                fg = wpool.tile([P, U], f32)
                for j in range(U):
                    nc.gpsimd.indirect_dma_start(
                        out=fg[:, j:j + 1],
                        out_offset=None,
                        in_=f_tab[:],
                        in_offset=bass.IndirectOffsetOnAxis(
                            ap=src_t2[:, j:j + 1], axis=0))
                wt = wpool.tile([P, U], f32)
                nc.vector.tensor_tensor(
                    out=wt[:],
                    in0=attr_t2[:].rearrange("p (j v) -> p j v", v=3)[:, :, 0],
                    in1=fg[:], op=mybir.AluOpType.mult)
                nc.sync.dma_start(w[i, :, :], wt[:])

    nc.compile()
    return nc, p


_CACHE = {}


def _get_full_kernel():
    key = "full"
    if key not in _CACHE:
        _CACHE[key] = build_kernel(**FULL)
    return _CACHE[key]


def _make_in_maps(inputs, p):
    NPAD = p["NPAD"]
    E_CORE = p["E_CORE"]
    vertex_attr = np.asarray(inputs["vertex_attr"], dtype=np.float32)
    edge_attr = np.ascontiguousarray(
        np.asarray(inputs["edge_attr"], dtype=np.float32))
    srcf = np.ascontiguousarray(
        np.asarray(inputs["edgeij_pair"], dtype=np.int32)[0])

    vpad = np.ones((NPAD, 2), dtype=np.float32)
    vpad[:vertex_attr.shape[0]] = vertex_attr

    in_maps = []
    for c in range(N_CORES):
        sl = slice(c * E_CORE, (c + 1) * E_CORE)
        outer = E_CORE // (P * FULL["u_tiles"])
        in_maps.append({
            "src": srcf[sl].reshape(outer, 128, FULL["u_tiles"]),
            "attr": edge_attr[sl].reshape(outer, 128, 3 * FULL["u_tiles"]),
            "vattr": vpad,
        })
    return in_maps


def kernel(vertex_attr, edge_attr, edgeij_pair):
    from concourse.bass_utils import run_bass_kernel_spmd

    nc, p = _get_full_kernel()
    in_maps = _make_in_maps(
        dict(vertex_attr=vertex_attr, edge_attr=edge_attr,
             edgeij_pair=edgeij_pair), p)
    res = run_bass_kernel_spmd(nc, in_maps, list(range(N_CORES)))
    return np.concatenate(
        [res.results[c]["w"].reshape(-1) for c in range(N_CORES)])



# revision 15
# speedup vs baseline: 6174.8665x; 6174.8665x over previous
"""Trainium2 Bass kernel for DirectInterpGNN message passing.

Math (per reference):
    num_v  = sum_{e: src_e=v} A_e
    den_v  = sum_{e: src_e=v} A_e*S_e*v_e
    f_v    = (C_v - 1) * (num_v/den_v) / A_ii_v
    w_e    = A_e * f_{src_e}

Sharding strategy (chosen; the hint's edge-split + all-reduce is one option,
this uses vertex-range sharding instead): edges are sorted by source vertex on
the host and split across the 8 cores at vertex boundaries, so each core owns
a disjoint contiguous vertex range and ALL edges incident to it. No collective
is needed.

Within a core's shard the host lays edges out in exact-degree classes: for
each per-core degree d, the n_d vertices of that degree occupy m_d = ceil(
n_d/128) slots per partition, their d edges contiguous in the free dimension.
The per-vertex segment sums then become plain innermost-axis tensor_reduce
calls, f is computed densely per vertex slot, and w_e = A_e * f is a single
broadcast multiply (0-stride AP along the degree axis). The device program is
fully dense: no indirect DMA, no dedup, no cross-partition traffic.

The device program's structure depends only on the class grid (list of
(d, m_d)) which is derived from the input degree histogram; compiled kernels
are cached by that grid.
"""
import sys
sys.path.insert(0, '/opt/trn_rl_repo')
sys.path.insert(0, '/root/.axon_site/_ro/trn_rl_repo')

import numpy as np
import ml_dtypes

P = 128
USE_BF16 = True
N_CORES = 8


# ---------------------------------------------------------------- host layout

def _build_layout(src, n_vertices):
    """Compute the sharding + degree-class layout for the given edge list.

    Returns a dict with the global class grid and per-core placement arrays.
    """
    E = src.shape[0]
    order = np.argsort(src, kind="stable")
    ssorted = src[order]

    # shard boundaries at vertex-run starts nearest to equal eighths
    bounds = [0]
    for c in range(1, N_CORES):
        t = (E * c) // N_CORES
        v = ssorted[t]
        b = int(np.searchsorted(ssorted, v, side="left"))
        bounds.append(b)
    bounds.append(E)

    cores = []
    for c in range(N_CORES):
        lo, hi = bounds[c], bounds[c + 1]
        seg_src = ssorted[lo:hi]
        vs, first, counts = np.unique(
            seg_src, return_index=True, return_counts=True)
        cores.append(dict(lo=lo, hi=hi, vs=vs, first=first, counts=counts))

    # global class grid: union of degrees, m_d = max over cores
    all_d = sorted({int(d) for core in cores for d in np.unique(core["counts"])})
    m_of = {}
    for d in all_d:
        m = 0
        for core in cores:
            n_d = int((core["counts"] == d).sum())
            m = max(m, -(-n_d // P))
        m_of[d] = m
    classes = [(d, m_of[d]) for d in all_d]
    FE = sum(d * m for d, m in classes)
    FV = sum(m for _, m in classes)
    eoff, voff = {}, {}
    e, v = 0, 0
    for d, m in classes:
        eoff[d] = e
        voff[d] = v
        e += d * m
        v += m

    # per-core placement
    for core in cores:
        vs, first, counts = core["vs"], core["first"], core["counts"]
        nv = len(vs)
        # order vertices by (degree, id); rank within class
        ordv = np.lexsort((vs, counts))
        rank = np.empty(nv, np.int64)
        d_ord = counts[ordv]
        grp_start = np.r_[0, np.flatnonzero(np.diff(d_ord)) + 1]
        within = np.arange(nv) - np.repeat(
            grp_start, np.diff(np.r_[grp_start, nv]))
        rank[ordv] = within
        p_v = rank % P
        i_v = rank // P
        dd = counts.astype(np.int64)
        off_e_v = np.array([eoff[int(d)] for d in dd], np.int64)
        off_v_v = np.array([voff[int(d)] for d in dd], np.int64)
        vcol = off_v_v + i_v
        ebase = off_e_v + i_v * dd
        # expand per edge
        j = np.arange(core["hi"] - core["lo"],
                      dtype=np.int64) - np.repeat(first, counts)
        part_e = np.repeat(p_v, counts)
        col_e = np.repeat(ebase, counts) + j
        core["flat_e"] = part_e * FE + col_e
        core["flat_v"] = p_v * FV + vcol
        core["edge_ids"] = order[core["lo"]:core["hi"]]

    key = tuple(classes)
    return dict(classes=classes, FE=FE, FV=FV, cores=cores, key=key, E=E)


def _make_in_maps(vertex_attr, edge_attr, layout, use_bf16=USE_BF16):
    edt = ml_dtypes.bfloat16 if use_bf16 else np.float32
    FE, FV = layout["FE"], layout["FV"]
    in_maps = []
    for core in layout["cores"]:
        ids = core["edge_ids"]
        fe = core["flat_e"]
        A = np.zeros(P * FE, edt)
        S = np.ones(P * FE, edt)
        V = np.ones(P * FE, edt)
        A[fe] = edge_attr[ids, 0].astype(edt)
        S[fe] = edge_attr[ids, 1].astype(edt)
        V[fe] = edge_attr[ids, 2].astype(edt)
        AII = np.ones(P * FV, np.float32)
        CC = np.ones(P * FV, np.float32)
        fv = core["flat_v"]
        AII[fv] = vertex_attr[core["vs"], 0]
        CC[fv] = vertex_attr[core["vs"], 1]
        in_maps.append({
            "ea": A.reshape(P, FE),
            "es": S.reshape(P, FE),
            "ev": V.reshape(P, FE),
            "vai": AII.reshape(P, FV),
            "vc": CC.reshape(P, FV),
        })
    return in_maps


# ------------------------------------------------------------- device program

def _make_pieces(classes, cw_max):
    """Split classes into (d, m_piece) pieces of width <= cw_max, then pack
    consecutive pieces into super-chunks of total width <= cw_max."""
    pieces = []
    for d, m in classes:
        assert d <= cw_max
        mrem = m
        while mrem:
            mw = min(mrem, max(1, cw_max // d))
            pieces.append((d, mw))
            mrem -= mw
    chunks = []
    cur, curw = [], 0
    for d, mw in pieces:
        w = d * mw
        if cur and curw + w > cw_max:
            chunks.append(cur)
            cur, curw = [], 0
        cur.append((d, mw))
        curw += w
    if cur:
        chunks.append(cur)
    return chunks


def build_kernel(classes, FE, FV, n_cores=N_CORES, repeats=1,
                 cw_max=3072, bufs=4, pool_mult=True, dma_split=True,
                 use_bf16=True, den_tree=False):
    from contextlib import nullcontext

    import concourse.bacc as bacc
    import concourse.mybir as mybir
    import concourse.tile as tile

    f32 = mybir.dt.float32
    bf16 = mybir.dt.bfloat16
    edt = bf16 if use_bf16 else f32
    mult = mybir.AluOpType.mult
    nc = bacc.Bacc("TRN2", target_bir_lowering=False, debug=False,
                   num_devices=n_cores)
    ea = nc.dram_tensor("ea", [P, FE], edt, kind="ExternalInput")
    es = nc.dram_tensor("es", [P, FE], edt, kind="ExternalInput")
    ev = nc.dram_tensor("ev", [P, FE], edt, kind="ExternalInput")
    vai = nc.dram_tensor("vai", [P, FV], f32, kind="ExternalInput")
    vc = nc.dram_tensor("vc", [P, FV], f32, kind="ExternalInput")
    w = nc.dram_tensor("w", [P, FE], edt, kind="ExternalOutput")

    chunks = _make_pieces(classes, cw_max)
    eng_es = nc.scalar if dma_split else nc.sync
    eng_ev = nc.scalar if dma_split else nc.sync
    eng_m = nc.gpsimd if pool_mult else nc.vector

    with tile.TileContext(nc) as tc:
        with (tc.tile_pool(name="const", bufs=1) as cpool,
              tc.tile_pool(name="stream", bufs=bufs) as spool,
              tc.tile_pool(name="small", bufs=bufs) as vpool,
              tc.For_i(0, repeats, 1) if repeats > 1 else nullcontext()):
            aii_t = cpool.tile([P, FV], f32)
            nc.sync.dma_start(aii_t[:], vai[:])
            # cm1 = C - 1, computed once for the whole vertex table
            cc_t = cpool.tile([P, FV], f32)
            nc.scalar.dma_start(cc_t[:], vc[:])
            cm1_t = cpool.tile([P, FV], f32)
            nc.vector.tensor_scalar(
                out=cm1_t[:], in0=cc_t[:], scalar1=-1.0, scalar2=None,
                op0=mybir.AluOpType.add)

            eo = 0
            vo = 0
            for chunk_i, chunk in enumerate(chunks):
                cw = sum(d * mw for d, mw in chunk)
                mw_tot = sum(mw for _, mw in chunk)
                a_t = spool.tile([P, cw], edt, tag="a")
                nc.sync.dma_start(a_t[:], ea[:, eo:eo + cw])
                s_t = spool.tile([P, cw], edt, tag="s")
                eng_es.dma_start(s_t[:], es[:, eo:eo + cw])
                v_t = spool.tile([P, cw], edt, tag="v")
                eng_ev.dma_start(v_t[:], ev[:, eo:eo + cw])
                # m = a*s*v, in place over s_t then v_t
                eng_m.tensor_tensor(out=s_t[:], in0=a_t[:], in1=s_t[:], op=mult)
                eng_m.tensor_tensor(out=v_t[:], in0=s_t[:], in1=v_t[:], op=mult)

                num_t = vpool.tile([P, mw_tot], f32, tag="num")
                den_t = vpool.tile([P, mw_tot], f32, tag="den")
                co = 0
                po = 0
                for d, mw in chunk:
                    if d > 1:
                        nc.vector.tensor_reduce(
                            out=num_t[:, po:po + mw],
                            in_=a_t[:, co:co + mw * d].rearrange(
                                "p (m d) -> p m d", d=d),
                            axis=mybir.AxisListType.X,
                            op=mybir.AluOpType.add)
                        if den_tree:
                            # pairwise in-place add-tree over the m tile on
                            # Pool; column 0 of each d-block ends up with den
                            bv = v_t[:, co:co + mw * d].rearrange(
                                "p (m d) -> p m d", d=d)
                            width = d
                            while width > 1:
                                h = width // 2
                                eng_m.tensor_tensor(
                                    out=bv[:, :, 0:h], in0=bv[:, :, 0:h],
                                    in1=bv[:, :, width - h:width],
                                    op=mybir.AluOpType.add)
                                width -= h
                            nc.vector.tensor_copy(
                                den_t[:, po:po + mw], bv[:, :, 0:1].rearrange(
                                    "p m o -> p (m o)"))
                        else:
                            nc.vector.tensor_reduce(
                                out=den_t[:, po:po + mw],
                                in_=v_t[:, co:co + mw * d].rearrange(
                                    "p (m d) -> p m d", d=d),
                                axis=mybir.AxisListType.X,
                                op=mybir.AluOpType.add)
                    else:
                        nc.vector.tensor_copy(
                            num_t[:, po:po + mw], a_t[:, co:co + mw])
                        nc.vector.tensor_copy(
                            den_t[:, po:po + mw], v_t[:, co:co + mw])
                    co += mw * d
                    po += mw

                # f = cm1 * num / (den_safe * A_ii) for the whole chunk
                dsafe = vpool.tile([P, mw_tot], f32, tag="dsafe")
                nc.vector.tensor_scalar(
                    out=dsafe[:], in0=den_t[:], scalar1=0.0, scalar2=None,
                    op0=mybir.AluOpType.is_equal)
                nc.vector.tensor_tensor(
                    out=dsafe[:], in0=dsafe[:], in1=den_t[:],
                    op=mybir.AluOpType.add)
                nc.vector.tensor_tensor(
                    out=dsafe[:], in0=dsafe[:], in1=aii_t[:, vo:vo + mw_tot],
                    op=mult)
                nc.vector.reciprocal(out=dsafe[:], in_=dsafe[:])
                nc.vector.tensor_tensor(
                    out=num_t[:], in0=num_t[:], in1=dsafe[:], op=mult)
                nc.vector.tensor_tensor(
                    out=num_t[:], in0=num_t[:], in1=cm1_t[:, vo:vo + mw_tot],
                    op=mult)

                # w = a * f (broadcast f along the degree axis), reuse s_t
                if use_bf16:
                    fb_t = vpool.tile([P, mw_tot], edt, tag="fb")
                    nc.vector.tensor_copy(fb_t[:], num_t[:])
                    f_src = fb_t
                else:
                    f_src = num_t
                co = 0
                po = 0
                for d, mw in chunk:
                    if d > 1:
                        f_b = f_src[:, po:po + mw].rearrange(
                            "p (m o) -> p m o", o=1).to_broadcast([P, mw, d])
                        eng_m.tensor_tensor(
                            out=s_t[:, co:co + mw * d].rearrange(
                                "p (m d) -> p m d", d=d),
                            in0=a_t[:, co:co + mw * d].rearrange(
                                "p (m d) -> p m d", d=d),
                            in1=f_b, op=mult)
                    else:
                        eng_m.tensor_tensor(
                            out=s_t[:, co:co + mw], in0=a_t[:, co:co + mw],
                            in1=f_src[:, po:po + mw], op=mult)
                    co += mw * d
                    po += mw
                eng_w = nc.sync if (chunk_i % 2 == 0) else nc.scalar
                eng_w.dma_start(w[:, eo:eo + cw], s_t[:])

                eo += cw
                vo += mw_tot
            assert eo == FE and vo == FV

    nc.compile()
    return nc


# ------------------------------------------------------------------- wrapper

_CACHE = {}


def _get_kernel(layout):
    key = (layout["key"], USE_BF16)
    if key not in _CACHE:
        _CACHE[key] = build_kernel(layout["classes"], layout["FE"],
                                   layout["FV"], use_bf16=USE_BF16)
    return _CACHE[key]


def kernel(vertex_attr, edge_attr, edgeij_pair):
    from concourse.bass_utils import run_bass_kernel_spmd

    vertex_attr = np.asarray(vertex_attr, dtype=np.float32)
    edge_attr = np.asarray(edge_attr, dtype=np.float32)
    src = np.ascontiguousarray(np.asarray(edgeij_pair, dtype=np.int32)[0])

    layout = _build_layout(src, vertex_attr.shape[0])
    nc = _get_kernel(layout)
    in_maps = _make_in_maps(vertex_attr, edge_attr, layout)
    res = run_bass_kernel_spmd(nc, in_maps, list(range(N_CORES)))

    out = np.empty(layout["E"], np.float32)
    for c, core in enumerate(layout["cores"]):
        wp = np.asarray(res.results[c]["w"]).astype(np.float32).reshape(-1)
        out[core["edge_ids"]] = wp[core["flat_e"]]
    return out
